# revision 36
# baseline (speedup 1.0000x reference)
"""Causal self-attention on 8 Trainium2 NeuronCores.

Sharding: tensor-parallel on heads. Each core owns 2 of the 16 heads
(128 of the 1024 feature dims), computes QKV projections for its heads,
full causal attention for its heads over all 4 batch elements, and a
row-parallel partial of the output projection. The 8 partial outputs
(f16) are summed on the host.

Layout strategy (everything contraction-dim-on-partitions):
  - x fed transposed: xT [C, B*T]
  - qT, kT computed as [hd, t] (hd = 2*64 local head dims stacked)
  - v transposed to [t, hd] via the XBAR DMA transpose (no PE/scalar
    involvement), with an appended ones-column per head for the
    softmax sums
  - ST tile = S^T = k @ q^T in [t_k, t_q] layout, so softmaxed P^T is
    directly the rhs of the PV matmul (no transposes in the hot loop)
  - the two heads' S matmuls occupy disjoint PE row groups (K=64 at
    base partitions 0/64) so they run concurrently in the array
  - S matmul + exp restricted to the un-masked causal column range
  - matmul data in fp16 (full PE rate, 2^-11 rel err); softmax
    denominators kept in f32; exp biased by -2 so fp16 never overflows
    (bias cancels exactly in softmax)
  - attention psum tiles are drained to sbuf immediately after the
    last PV (f16 raw copies) so the next block's PV can start while
    the reciprocal+scale (vector + gpsimd) finish off-path
  - causal masks on gpsimd; exp on scalar; drains/normalize/output
    casts on vector; out-proj overlaps the next block's kc loop via a
    2-deep psum rotation
"""

import json

import numpy as np

import concourse.bass as bass
import concourse.mybir as mybir
import concourse.tile as tile
import concourse.bass2jax as bass2jax
import concourse.bass_utils as bass_utils
from concourse.bass import ts
from concourse.masks import make_identity, make_upper_triangular

B, T, C, H, D = 4, 2048, 1024, 16, 64
NCORES = 8
HL = H // NCORES          # heads per core = 2
HD = HL * D               # local head dims = 128
TF = B * T                # flattened tokens = 8192
NKC = C // 128            # contraction chunks for projections = 8
NTB = TF // 512           # 512-wide token blocks = 16
QB = 512                  # q block width
NQB = T // QB             # q blocks per batch elem = 4
TKC = T // 128            # 128-wide k chunks per batch elem = 16

f32 = mybir.dt.float32
f16 = mybir.dt.float16
EXP = mybir.ActivationFunctionType.Exp
IDENT_FN = mybir.ActivationFunctionType.Identity
EXP_BIAS = -2.0           # exp(s - 2): keeps exp outputs well inside fp16

NP16 = np.float16


# --- workaround: this walrus build accepts at most one sync wait per
# instruction; Tile's final drain carries one wait per outstanding proc.
# Hoist surplus waits onto single-wait drain carriers in the BIR json.
_orig_compile_bir_kernel = None


# this walrus build accepts exactly one sync wait on every instruction
MAX_WAITS_COMPUTE = 1
MAX_WAITS_CTRL = 1


def _split_waits_in_bir(bir_json):
    d = json.loads(bir_json)
    n = 0
    for f in d.get("functions", []):
        for bb in f.get("blocks", []):
            insts = bb.get("instructions", [])
            new_insts = []
            for inst in insts:
                si = inst.get("sync_info") or {}
                waits = si.get("on_wait") or []
                limit = (
                    MAX_WAITS_CTRL
                    if inst["opcode"]
                    in ("Drain", "EventSemaphore", "NoOp", "DMACopy", "DMA")
                    else MAX_WAITS_COMPUTE
                )
                if len(waits) > limit:
                    surplus = waits[:-limit]
                    for k, w in enumerate(surplus):
                        new_insts.append({
                            "name": f"{inst['name']}_wsplit{k}",
                            "engine": inst["engine"],
                            "opcode": "EventSemaphore",
                            "ins": [],
                            "outs": [],
                            "debug": inst.get("debug", 0),
                            "sync_info": {"on_update": [], "on_wait": [w]},
                        })
                        n += 1
                    si["on_wait"] = waits[-limit:]
                    inst["sync_info"] = si
                new_insts.append(inst)
            bb["instructions"] = new_insts
    return json.dumps(d).encode()


def _install_wait_split():
    global _orig_compile_bir_kernel
    if _orig_compile_bir_kernel is not None:
        return
    _orig_compile_bir_kernel = bass2jax.compile_bir_kernel

    def _patched(bir_json, tmpdir, neff_name="file.neff"):
        return _orig_compile_bir_kernel(
            _split_waits_in_bir(bir_json), tmpdir, neff_name
        )

    bass2jax.compile_bir_kernel = _patched


def build_program():
    nc = bass.Bass()
    xT = nc.declare_dram_parameter("xT", [C, TF], f16, isOutput=False)
    wqkvT = nc.declare_dram_parameter("wqkvT", [C, 3 * HD], f16, isOutput=False)
    wpT = nc.declare_dram_parameter("wpT", [HD, C], f16, isOutput=False)
    bqkv = nc.declare_dram_parameter("bqkv", [HD, 3], f32, isOutput=False)
    outT = nc.declare_dram_parameter("outT", [C, TF], f16, isOutput=True)

    with tile.TileContext(nc) as tc:
        with (
            tc.tile_pool(name="consts", bufs=1) as consts,
            tc.tile_pool(name="persist", bufs=1) as persist,
        ):
            ident = consts.tile([128, 128], f16)
            make_identity(nc, ident)
            tri = consts.tile([128, 128], f16)
            make_upper_triangular(nc, tri, val=1.0, diag=True)
            # ones row for the K=1 denominator-broadcast matmuls
            ones64 = consts.tile([1, 64], f16)
            nc.vector.memset(ones64, 1.0)
            expbias = consts.tile([128, 1], f32)
            nc.vector.memset(expbias, EXP_BIAS)

            wq_sb = consts.tile([128, NKC, 3 * HD], f16)
            nc.sync.dma_start(wq_sb, wqkvT.rearrange("(kc p) n -> p kc n", p=128))
            wp_sb = consts.tile([HD, C], f16)
            nc.sync.dma_start(wp_sb, wpT[:, :])
            b_sb = consts.tile([HD, 3], f32)
            nc.sync.dma_start(b_sb, bqkv[:, :])

            qT = persist.tile([128, TF], f16)
            kT = persist.tile([128, TF], f16)
            yT = persist.tile([128, TF], f16)
            # v in [t, hd] layout + a ones column per head for softmax sums
            v_sb = persist.tile([128, B, TKC, HL, 66], f16)
            nc.vector.memset(v_sb[:, :, :, :, 64], 1.0)

            xTr = xT.rearrange("(kc p) t -> p kc t", p=128)
            outTr = outT.rearrange("(r p) t -> p r t", p=128)

            # ---- phase 1: QKV projections (+ v transposed via XBAR DMA) ----
            with (
                tc.tile_pool(name="p1", bufs=3) as p1,
                tc.tile_pool(name="ps1", bufs=1, space="PSUM") as ps1,
            ):
                for tb in range(NTB):
                    tsl = ts(tb, 512)
                    # one 1MB DMA brings all 8 contraction chunks
                    xt = p1.tile([128, NKC, 512], f16, tag="xt", name="xt")
                    nc.sync.dma_start(xt, xTr[:, :, tsl])
                    # K-contiguous accumulation per projection; psum
                    # drains on the scalar engine (idle in phase 1)
                    for pr in range(3):
                        pst_t = ps1.tile([128, 512], f32, tag="qkvps",
                                         bufs=2, name="qkvps")
                        for kc in range(NKC):
                            nc.tensor.matmul(
                                pst_t,
                                lhsT=wq_sb[:, kc, ts(pr, HD)],
                                rhs=xt[:, kc, :],
                                start=(kc == 0),
                                stop=(kc == NKC - 1),
                            )
                        if pr == 0:
                            nc.scalar.activation(
                                qT[:, tsl], pst_t, IDENT_FN, bias=b_sb[:, 0:1]
                            )
                        elif pr == 1:
                            nc.scalar.activation(
                                kT[:, tsl], pst_t, IDENT_FN, bias=b_sb[:, 1:2]
                            )
                        else:
                            vt = p1.tile([128, 512], f16, tag="vt", name="vt")
                            nc.scalar.activation(
                                vt, pst_t, IDENT_FN, bias=b_sb[:, 2:3]
                            )
                            for i in range(4):
                                b_i, kc_i = divmod(tb * 4 + i, TKC)
                                pt = ps1.tile([128, 128], f16, tag="vtp",
                                              bufs=2, name="vtp")
                                nc.tensor.transpose(
                                    pt, vt[:, ts(i, 128)], ident
                                )
                                nc.scalar.activation(
                                    v_sb[:, b_i, kc_i, :, 0:64],
                                    pt[:, :].rearrange(
                                        "p (h d) -> p h d", h=HL
                                    ),
                                    IDENT_FN,
                                )

            # ---- phase 2: causal attention + output projection ----
            with (
                tc.tile_pool(name="p2", bufs=3) as p2,
                tc.tile_pool(name="ps2", bufs=1, space="PSUM") as ps2,
            ):
                # deferred normalize-broadcast + out-proj steps of the
                # previous block, emitted inside the next block's kc
                # loop: by then their inputs are ready, so they fill
                # exp-paced PE bubbles instead of stalling the strict
                # engine FIFOs at block boundaries
                pending = []

                def drain_pending(n):
                    for _ in range(min(n, len(pending))):
                        pending.pop(0)()

                # blocks run j-descending: the long j=3 block leads each
                # batch, giving the machine ~16 kc-steps of exp work to
                # absorb the previous batch's deferred tail
                for b_i in range(B):
                    for j in reversed(range(NQB)):
                        q_off = b_i * T + j * QB
                        qsl = slice(q_off, q_off + QB)
                        ypq = [
                            ps2.tile([65, 512], f32, tag=f"y{h}", bufs=1,
                                     name=f"ypq{h}")
                            for h in range(HL)
                        ]
                        nkc = 4 * (j + 1)

                        def emit_s(kc):
                            # both heads' S^T tiles side by side in one
                            # 2-bank psum tile -> one exp op over both;
                            # the two K=64 matmuls use disjoint PE row
                            # groups (base partitions 0/64) so they run
                            # concurrently. Columns < lo are fully
                            # masked -> never computed.
                            r = kc * 128 - j * QB
                            k_off = b_i * T + kc * 128
                            lo = max(r, 0)
                            st = ps2.tile([128, 2, 512], f32, tag="st",
                                          bufs=2, name="st")
                            for h in range(HL):
                                nc.tensor.matmul(
                                    st[:, h, lo:512],
                                    lhsT=kT[ts(h, 64), k_off:k_off + 128],
                                    rhs=qT[ts(h, 64), q_off + lo:q_off + 512],
                                    start=True,
                                    stop=True,
                                )
                            return st, lo, r

                        # software pipeline: S runs two kc-steps ahead
                        # of exp/PV so the exp stream never waits on the
                        # S->exp->PV->S round trip (PVs lag behind,
                        # buffered by the ex pool)
                        window = [emit_s(kc) for kc in range(min(2, nkc))]
                        for kc in range(nkc):
                            st, lo, r = window.pop(0)
                            ex = p2.tile([128, 2, 512], f16, tag="ex",
                                         bufs=6, name="ex")
                            nc.scalar.activation(
                                ex[:, :, lo:512], st[:, :, lo:512], EXP,
                                scale=0.125, bias=expbias,
                            )
                            if kc + 2 < nkc:
                                window.append(emit_s(kc + 2))
                            for h in range(HL):
                                if r >= 0:
                                    # causal mask on gpsimd (idle engine)
                                    nc.gpsimd.tensor_mul(
                                        ex[:, h, r:r + 128],
                                        ex[:, h, r:r + 128], tri
                                    )
                                nc.tensor.matmul(
                                    ypq[h][:, lo:512],
                                    lhsT=v_sb[:, b_i, kc, h, 0:65],
                                    rhs=ex[:, h, lo:512],
                                    start=(kc == 0),
                                    stop=(kc == nkc - 1),
                                )
                            # the norm closure (start of the recip
                            # chain) drains immediately; projection
                            # closures wait until kc>=3, by which point
                            # yT of the previous block is ready and the
                            # pp matmuls no longer stall the PE FIFO
                            drain_pending(1 if kc <= 1 else
                                          (2 if kc >= 3 else 0))
                        # normalize: yT[hd, t] = yT_unnorm * (1/sumexp).
                        # ypq is drained to sbuf (f16) immediately so
                        # its psum banks free up for the next block; the
                        # reciprocal+scale runs off the psum critical
                        # path (scale on gpsimd, sbuf-only).
                        srow = p2.tile([1, 1024], f16, tag="srow", name="srow")
                        for h in range(HL):
                            nc.vector.tensor_copy(
                                srow[0:1, ts(h, 512)], ypq[h][64:65, :]
                            )
                        yraw = p2.tile([128, 512], f16, tag="yraw", name="yraw")
                        for h in range(HL):
                            nc.vector.tensor_copy(
                                yraw[ts(h, 64), :], ypq[h][0:64, :]
                            )

                        def emit_norm(srow=srow, yraw=yraw, qsl=qsl):
                            # two K=1 matmuls to disjoint PE col groups
                            # replicate each head's sums row across its
                            # 64 output partitions; one reciprocal
                            # covers both heads
                            bc = ps2.tile([128, 512], f32, tag="misc",
                                          bufs=2, name="bc")
                            for h in range(HL):
                                nc.tensor.matmul(
                                    bc[ts(h, 64), :], lhsT=ones64,
                                    rhs=srow[0:1, ts(h, 512)],
                                    start=True, stop=True,
                                )
                            rec = p2.tile([128, 512], f16, tag="rec",
                                          name="rec")
                            with nc.allow_low_precision(
                                reason="softmax 1/sum applied in f16; "
                                "2^-11 rel err is well inside the "
                                "output tolerance"
                            ):
                                nc.vector.reciprocal(rec, bc)
                                nc.gpsimd.tensor_mul(yT[:, qsl], yraw, rec)

                        def emit_proj(oc0, qsl=qsl):
                            # row-parallel output projection, two
                            # 128-row chunks per step
                            for oc in (oc0, oc0 + 1):
                                pp = ps2.tile([128, 512], f32, tag="misc",
                                              bufs=2, name="pp")
                                nc.tensor.matmul(
                                    pp,
                                    lhsT=wp_sb[:, ts(oc, 128)],
                                    rhs=yT[:, qsl],
                                    start=True,
                                    stop=True,
                                )
                                ob = p2.tile([128, 512], f16, tag="ob",
                                             bufs=6, name="ob")
                                # casts stay off the scalar engine: with
                                # the deferred tail they would insert
                                # between back-to-back exps in the
                                # scalar FIFO and pace down the exp
                                # stream
                                nc.vector.tensor_copy(ob, pp)
                                nc.sync.dma_start(outT[ts(oc, 128), qsl], ob)

                        pending.append(emit_norm)
                        for oc0 in range(0, 8, 2):
                            pending.append(
                                lambda oc0=oc0, f=emit_proj: f(oc0)
                            )
                # flush the final block's deferred tail
                drain_pending(len(pending))
    return nc


_program = None


def _get_program():
    global _program
    if _program is None:
        _install_wait_split()
        _program = build_program()
    return _program


def kernel(x, Wq, bq, Wk, bk, Wv, bv, Wp, bp):
    nc = _get_program()

    x = np.asarray(x, dtype=np.float32)
    xT = np.ascontiguousarray(x.reshape(TF, C).T.astype(NP16))
    in_maps = []
    for core in range(NCORES):
        rows = slice(core * HD, (core + 1) * HD)
        wqkvT = np.ascontiguousarray(
            np.concatenate(
                [np.asarray(W, np.float32)[rows].T for W in (Wq, Wk, Wv)], axis=1
            ).astype(NP16)
        )
        wpT = np.ascontiguousarray(np.asarray(Wp, np.float32)[:, rows].T.astype(NP16))
        bq_l = np.stack(
            [np.asarray(v, np.float32)[rows] for v in (bq, bk, bv)], axis=1
        )
        in_maps.append(
            {
                "xT": xT,
                "wqkvT": wqkvT,
                "wpT": wpT,
                "bqkv": np.ascontiguousarray(bq_l),
            }
        )

    r = bass_utils.run_bass_kernel_spmd(nc, in_maps, list(range(NCORES)))
    acc = r.results[0]["outT"].astype(np.float32)
    for core in range(1, NCORES):
        acc = acc + r.results[core]["outT"].astype(np.float32)
    out = acc.T.reshape(B, T, C) + np.asarray(bp, np.float32)[None, None, :]
    return out.astype(np.float32)


# revision 37
# speedup vs baseline: 1.0432x; 1.0432x over previous
"""Causal self-attention on 8 Trainium2 NeuronCores.

Sharding: tensor-parallel on heads. Each core owns 2 of the 16 heads
(128 of the 1024 feature dims), computes QKV projections for its heads,
full causal attention for its heads over all 4 batch elements, and a
row-parallel partial of the output projection. The 8 partial outputs
(f16) are summed on the host.

Layout strategy (everything contraction-dim-on-partitions):
  - x fed transposed: xT [C, B*T]
  - qT, kT computed as [hd, t] (hd = 2*64 local head dims stacked)
  - v transposed to [t, hd] via the XBAR DMA transpose (no PE/scalar
    involvement), with an appended ones-column per head for the
    softmax sums
  - ST tile = S^T = k @ q^T in [t_k, t_q] layout, so softmaxed P^T is
    directly the rhs of the PV matmul (no transposes in the hot loop)
  - the two heads' S matmuls occupy disjoint PE row groups (K=64 at
    base partitions 0/64) so they run concurrently in the array
  - S matmul + exp restricted to the un-masked causal column range
  - matmul data in fp16 (full PE rate, 2^-11 rel err); softmax
    denominators kept in f32; exp biased by -2 so fp16 never overflows
    (bias cancels exactly in softmax)
  - attention psum tiles are drained to sbuf immediately after the
    last PV (f16 raw copies) so the next block's PV can start while
    the reciprocal+scale (vector + gpsimd) finish off-path
  - causal masks on gpsimd; exp on scalar; drains/normalize/output
    casts on vector; out-proj overlaps the next block's kc loop via a
    2-deep psum rotation
"""

import json

import numpy as np

import concourse.bass as bass
import concourse.mybir as mybir
import concourse.tile as tile
import concourse.bass2jax as bass2jax
import concourse.bass_utils as bass_utils
from concourse.bass import ts
from concourse.masks import make_identity, make_upper_triangular

B, T, C, H, D = 4, 2048, 1024, 16, 64
NCORES = 8
HL = H // NCORES          # heads per core = 2
HD = HL * D               # local head dims = 128
TF = B * T                # flattened tokens = 8192
NKC = C // 128            # contraction chunks for projections = 8
NTB = TF // 512           # 512-wide token blocks = 16
QB = 512                  # q block width
NQB = T // QB             # q blocks per batch elem = 4
TKC = T // 128            # 128-wide k chunks per batch elem = 16

f32 = mybir.dt.float32
f16 = mybir.dt.float16
EXP = mybir.ActivationFunctionType.Exp
IDENT_FN = mybir.ActivationFunctionType.Identity
EXP_BIAS = -2.0           # exp(s - 2): keeps exp outputs well inside fp16

NP16 = np.float16


# --- workaround: this walrus build accepts at most one sync wait per
# instruction; Tile's final drain carries one wait per outstanding proc.
# Hoist surplus waits onto single-wait drain carriers in the BIR json.
_orig_compile_bir_kernel = None


# this walrus build accepts exactly one sync wait on every instruction
MAX_WAITS_COMPUTE = 1
MAX_WAITS_CTRL = 1


def _split_waits_in_bir(bir_json):
    d = json.loads(bir_json)
    n = 0
    for f in d.get("functions", []):
        for bb in f.get("blocks", []):
            insts = bb.get("instructions", [])
            new_insts = []
            for inst in insts:
                si = inst.get("sync_info") or {}
                waits = si.get("on_wait") or []
                limit = (
                    MAX_WAITS_CTRL
                    if inst["opcode"]
                    in ("Drain", "EventSemaphore", "NoOp", "DMACopy", "DMA")
                    else MAX_WAITS_COMPUTE
                )
                if len(waits) > limit:
                    surplus = waits[:-limit]
                    for k, w in enumerate(surplus):
                        new_insts.append({
                            "name": f"{inst['name']}_wsplit{k}",
                            "engine": inst["engine"],
                            "opcode": "EventSemaphore",
                            "ins": [],
                            "outs": [],
                            "debug": inst.get("debug", 0),
                            "sync_info": {"on_update": [], "on_wait": [w]},
                        })
                        n += 1
                    si["on_wait"] = waits[-limit:]
                    inst["sync_info"] = si
                new_insts.append(inst)
            bb["instructions"] = new_insts
    return json.dumps(d).encode()


def _install_wait_split():
    global _orig_compile_bir_kernel
    if _orig_compile_bir_kernel is not None:
        return
    _orig_compile_bir_kernel = bass2jax.compile_bir_kernel

    def _patched(bir_json, tmpdir, neff_name="file.neff"):
        return _orig_compile_bir_kernel(
            _split_waits_in_bir(bir_json), tmpdir, neff_name
        )

    bass2jax.compile_bir_kernel = _patched


def build_program():
    nc = bass.Bass()
    xT = nc.declare_dram_parameter("xT", [C, TF], f16, isOutput=False)
    wqkvT = nc.declare_dram_parameter("wqkvT", [C, 3 * HD], f16, isOutput=False)
    wpT = nc.declare_dram_parameter("wpT", [HD, C], f16, isOutput=False)
    bqkv = nc.declare_dram_parameter("bqkv", [HD, 3], f32, isOutput=False)
    outT = nc.declare_dram_parameter("outT", [C, TF], f16, isOutput=True)

    with tile.TileContext(nc) as tc:
        with (
            tc.tile_pool(name="consts", bufs=1) as consts,
            tc.tile_pool(name="persist", bufs=1) as persist,
        ):
            ident = consts.tile([128, 128], f16)
            make_identity(nc, ident)
            tri = consts.tile([128, 128], f16)
            make_upper_triangular(nc, tri, val=1.0, diag=True)
            # ones row for the K=1 denominator-broadcast matmuls
            ones64 = consts.tile([1, 64], f16)
            nc.vector.memset(ones64, 1.0)
            expbias = consts.tile([128, 1], f32)
            nc.vector.memset(expbias, EXP_BIAS)

            wq_sb = consts.tile([128, NKC, 3 * HD], f16)
            nc.sync.dma_start(wq_sb, wqkvT.rearrange("(kc p) n -> p kc n", p=128))
            wp_sb = consts.tile([HD, C], f16)
            nc.sync.dma_start(wp_sb, wpT[:, :])
            b_sb = consts.tile([HD, 3], f32)
            nc.sync.dma_start(b_sb, bqkv[:, :])

            qT = persist.tile([128, TF], f16)
            kT = persist.tile([128, TF], f16)
            yT = persist.tile([128, TF], f16)
            # v in [t, hd] layout + a ones column per head for softmax sums
            v_sb = persist.tile([128, B, TKC, HL, 66], f16)
            nc.vector.memset(v_sb[:, :, :, :, 64], 1.0)

            xTr = xT.rearrange("(kc p) t -> p kc t", p=128)
            outTr = outT.rearrange("(r p) t -> p r t", p=128)

            # ---- phase 1: QKV projections (+ v transposed via XBAR DMA) ----
            with (
                tc.tile_pool(name="p1", bufs=3) as p1,
                tc.tile_pool(name="ps1", bufs=1, space="PSUM") as ps1,
            ):
                for tb in range(NTB):
                    tsl = ts(tb, 512)
                    # one 1MB DMA brings all 8 contraction chunks
                    xt = p1.tile([128, NKC, 512], f16, tag="xt", name="xt")
                    nc.sync.dma_start(xt, xTr[:, :, tsl])
                    # K-contiguous accumulation per projection; psum
                    # drains on the scalar engine (idle in phase 1)
                    for pr in range(3):
                        pst_t = ps1.tile([128, 512], f32, tag="qkvps",
                                         bufs=4, name="qkvps")
                        for kc in range(NKC):
                            nc.tensor.matmul(
                                pst_t,
                                lhsT=wq_sb[:, kc, ts(pr, HD)],
                                rhs=xt[:, kc, :],
                                start=(kc == 0),
                                stop=(kc == NKC - 1),
                            )
                        if pr == 0:
                            nc.scalar.activation(
                                qT[:, tsl], pst_t, IDENT_FN, bias=b_sb[:, 0:1]
                            )
                        elif pr == 1:
                            nc.scalar.activation(
                                kT[:, tsl], pst_t, IDENT_FN, bias=b_sb[:, 1:2]
                            )
                        else:
                            vt = p1.tile([128, 512], f16, tag="vt", name="vt")
                            nc.scalar.activation(
                                vt, pst_t, IDENT_FN, bias=b_sb[:, 2:3]
                            )
                            for i in range(4):
                                b_i, kc_i = divmod(tb * 4 + i, TKC)
                                pt = ps1.tile([128, 128], f16, tag="vtp",
                                              bufs=2, name="vtp")
                                nc.tensor.transpose(
                                    pt, vt[:, ts(i, 128)], ident
                                )
                                nc.scalar.activation(
                                    v_sb[:, b_i, kc_i, :, 0:64],
                                    pt[:, :].rearrange(
                                        "p (h d) -> p h d", h=HL
                                    ),
                                    IDENT_FN,
                                )

            # ---- phase 2: causal attention + output projection ----
            with (
                tc.tile_pool(name="p2", bufs=3) as p2,
                tc.tile_pool(name="ps2", bufs=1, space="PSUM") as ps2,
            ):
                # deferred normalize-broadcast + out-proj steps of the
                # previous block, emitted inside the next block's kc
                # loop: by then their inputs are ready, so they fill
                # exp-paced PE bubbles instead of stalling the strict
                # engine FIFOs at block boundaries
                pending = []

                def drain_pending(n):
                    for _ in range(min(n, len(pending))):
                        pending.pop(0)()

                # blocks run j-descending: the long j=3 block leads each
                # batch, giving the machine ~16 kc-steps of exp work to
                # absorb the previous batch's deferred tail
                for b_i in range(B):
                    for j in reversed(range(NQB)):
                        q_off = b_i * T + j * QB
                        qsl = slice(q_off, q_off + QB)
                        ypq = [
                            ps2.tile([65, 512], f32, tag=f"y{h}", bufs=1,
                                     name=f"ypq{h}")
                            for h in range(HL)
                        ]
                        nkc = 4 * (j + 1)

                        def emit_s(kc):
                            # both heads' S^T tiles side by side in one
                            # 2-bank psum tile -> one exp op over both;
                            # the two K=64 matmuls use disjoint PE row
                            # groups (base partitions 0/64) so they run
                            # concurrently. Columns < lo are fully
                            # masked -> never computed.
                            r = kc * 128 - j * QB
                            k_off = b_i * T + kc * 128
                            lo = max(r, 0)
                            st = ps2.tile([128, 2, 512], f32, tag="st",
                                          bufs=2, name="st")
                            for h in range(HL):
                                nc.tensor.matmul(
                                    st[:, h, lo:512],
                                    lhsT=kT[ts(h, 64), k_off:k_off + 128],
                                    rhs=qT[ts(h, 64), q_off + lo:q_off + 512],
                                    start=True,
                                    stop=True,
                                )
                            return st, lo, r

                        # software pipeline: S runs two kc-steps ahead
                        # of exp/PV so the exp stream never waits on the
                        # S->exp->PV->S round trip (PVs lag behind,
                        # buffered by the ex pool)
                        window = [emit_s(kc) for kc in range(min(2, nkc))]
                        for kc in range(nkc):
                            st, lo, r = window.pop(0)
                            ex = p2.tile([128, 2, 512], f16, tag="ex",
                                         bufs=6, name="ex")
                            nc.scalar.activation(
                                ex[:, :, lo:512], st[:, :, lo:512], EXP,
                                scale=0.125, bias=expbias,
                            )
                            if kc + 2 < nkc:
                                window.append(emit_s(kc + 2))
                            for h in range(HL):
                                if r >= 0:
                                    # causal mask on gpsimd (idle engine)
                                    nc.gpsimd.tensor_mul(
                                        ex[:, h, r:r + 128],
                                        ex[:, h, r:r + 128], tri
                                    )
                                nc.tensor.matmul(
                                    ypq[h][:, lo:512],
                                    lhsT=v_sb[:, b_i, kc, h, 0:65],
                                    rhs=ex[:, h, lo:512],
                                    start=(kc == 0),
                                    stop=(kc == nkc - 1),
                                )
                            # the norm closure (start of the recip
                            # chain) drains immediately; projection
                            # closures wait until kc>=3, by which point
                            # yT of the previous block is ready and the
                            # pp matmuls no longer stall the PE FIFO
                            drain_pending(1 if kc == 0 else
                                          (2 if kc >= 4 else 0))
                        # normalize: yT[hd, t] = yT_unnorm * (1/sumexp).
                        # ypq is drained to sbuf (f16) immediately so
                        # its psum banks free up for the next block; the
                        # reciprocal+scale runs off the psum critical
                        # path (scale on gpsimd, sbuf-only).
                        srow = p2.tile([1, 1024], f16, tag="srow", name="srow")
                        for h in range(HL):
                            nc.vector.tensor_copy(
                                srow[0:1, ts(h, 512)], ypq[h][64:65, :]
                            )
                        yraw = p2.tile([128, 512], f16, tag="yraw", name="yraw")
                        for h in range(HL):
                            nc.vector.tensor_copy(
                                yraw[ts(h, 64), :], ypq[h][0:64, :]
                            )

                        def emit_norm(srow=srow, yraw=yraw, qsl=qsl):
                            # two K=1 matmuls to disjoint PE col groups
                            # replicate each head's sums row across its
                            # 64 output partitions; one reciprocal
                            # covers both heads
                            bc = ps2.tile([128, 512], f32, tag="misc",
                                          bufs=2, name="bc")
                            for h in range(HL):
                                nc.tensor.matmul(
                                    bc[ts(h, 64), :], lhsT=ones64,
                                    rhs=srow[0:1, ts(h, 512)],
                                    start=True, stop=True,
                                )
                            rec = p2.tile([128, 512], f16, tag="rec",
                                          name="rec")
                            with nc.allow_low_precision(
                                reason="softmax 1/sum applied in f16; "
                                "2^-11 rel err is well inside the "
                                "output tolerance"
                            ):
                                nc.vector.reciprocal(rec, bc)
                                nc.gpsimd.tensor_mul(yT[:, qsl], yraw, rec)

                        def emit_proj(oc0, qsl=qsl):
                            # row-parallel output projection, two
                            # 128-row chunks per step
                            for oc in (oc0, oc0 + 1):
                                pp = ps2.tile([128, 512], f32, tag="misc",
                                              bufs=2, name="pp")
                                nc.tensor.matmul(
                                    pp,
                                    lhsT=wp_sb[:, ts(oc, 128)],
                                    rhs=yT[:, qsl],
                                    start=True,
                                    stop=True,
                                )
                                ob = p2.tile([128, 512], f16, tag="ob",
                                             bufs=6, name="ob")
                                # casts stay off the scalar engine: with
                                # the deferred tail they would insert
                                # between back-to-back exps in the
                                # scalar FIFO and pace down the exp
                                # stream
                                nc.vector.tensor_copy(ob, pp)
                                nc.sync.dma_start(outT[ts(oc, 128), qsl], ob)

                        pending.append(emit_norm)
                        for oc0 in range(0, 8, 2):
                            pending.append(
                                lambda oc0=oc0, f=emit_proj: f(oc0)
                            )
                # flush the final block's deferred tail
                drain_pending(len(pending))
    return nc


_program = None


def _get_program():
    global _program
    if _program is None:
        _install_wait_split()
        _program = build_program()
    return _program


def kernel(x, Wq, bq, Wk, bk, Wv, bv, Wp, bp):
    nc = _get_program()

    x = np.asarray(x, dtype=np.float32)
    xT = np.ascontiguousarray(x.reshape(TF, C).T.astype(NP16))
    in_maps = []
    for core in range(NCORES):
        rows = slice(core * HD, (core + 1) * HD)
        wqkvT = np.ascontiguousarray(
            np.concatenate(
                [np.asarray(W, np.float32)[rows].T for W in (Wq, Wk, Wv)], axis=1
            ).astype(NP16)
        )
        wpT = np.ascontiguousarray(np.asarray(Wp, np.float32)[:, rows].T.astype(NP16))
        bq_l = np.stack(
            [np.asarray(v, np.float32)[rows] for v in (bq, bk, bv)], axis=1
        )
        in_maps.append(
            {
                "xT": xT,
                "wqkvT": wqkvT,
                "wpT": wpT,
                "bqkv": np.ascontiguousarray(bq_l),
            }
        )

    r = bass_utils.run_bass_kernel_spmd(nc, in_maps, list(range(NCORES)))
    acc = r.results[0]["outT"].astype(np.float32)
    for core in range(1, NCORES):
        acc = acc + r.results[core]["outT"].astype(np.float32)
    out = acc.T.reshape(B, T, C) + np.asarray(bp, np.float32)[None, None, :]
    return out.astype(np.float32)


# revision 38
# speedup vs baseline: 1.0619x; 1.0179x over previous
"""Causal self-attention on 8 Trainium2 NeuronCores.

Sharding: tensor-parallel on heads. Each core owns 2 of the 16 heads
(128 of the 1024 feature dims), computes QKV projections for its heads,
full causal attention for its heads over all 4 batch elements, and a
row-parallel partial of the output projection. The 8 partial outputs
(f16) are summed on the host.

Layout strategy (everything contraction-dim-on-partitions):
  - x fed transposed: xT [C, B*T]
  - qT, kT computed as [hd, t] (hd = 2*64 local head dims stacked)
  - v transposed to [t, hd] via the XBAR DMA transpose (no PE/scalar
    involvement), with an appended ones-column per head for the
    softmax sums
  - ST tile = S^T = k @ q^T in [t_k, t_q] layout, so softmaxed P^T is
    directly the rhs of the PV matmul (no transposes in the hot loop)
  - the two heads' S matmuls occupy disjoint PE row groups (K=64 at
    base partitions 0/64) so they run concurrently in the array
  - S matmul + exp restricted to the un-masked causal column range
  - matmul data in fp16 (full PE rate, 2^-11 rel err); softmax
    denominators kept in f32; exp biased by -2 so fp16 never overflows
    (bias cancels exactly in softmax)
  - attention psum tiles are drained to sbuf immediately after the
    last PV (f16 raw copies) so the next block's PV can start while
    the reciprocal+scale (vector + gpsimd) finish off-path
  - causal masks on gpsimd; exp on scalar; drains/normalize/output
    casts on vector; out-proj overlaps the next block's kc loop via a
    2-deep psum rotation
"""

import json

import numpy as np

import concourse.bass as bass
import concourse.mybir as mybir
import concourse.tile as tile
import concourse.bass2jax as bass2jax
import concourse.bass_utils as bass_utils
from concourse.bass import ts
from concourse.masks import make_identity, make_upper_triangular

B, T, C, H, D = 4, 2048, 1024, 16, 64
NCORES = 8
HL = H // NCORES          # heads per core = 2
HD = HL * D               # local head dims = 128
TF = B * T                # flattened tokens = 8192
NKC = C // 128            # contraction chunks for projections = 8
NTB = TF // 512           # 512-wide token blocks = 16
QB = 512                  # q block width
NQB = T // QB             # q blocks per batch elem = 4
TKC = T // 128            # 128-wide k chunks per batch elem = 16

f32 = mybir.dt.float32
f16 = mybir.dt.float16
EXP = mybir.ActivationFunctionType.Exp
IDENT_FN = mybir.ActivationFunctionType.Identity
EXP_BIAS = -2.0           # exp(s - 2): keeps exp outputs well inside fp16

NP16 = np.float16


# --- workaround: this walrus build accepts at most one sync wait per
# instruction; Tile's final drain carries one wait per outstanding proc.
# Hoist surplus waits onto single-wait drain carriers in the BIR json.
_orig_compile_bir_kernel = None


# this walrus build accepts exactly one sync wait on every instruction
MAX_WAITS_COMPUTE = 1
MAX_WAITS_CTRL = 1


def _split_waits_in_bir(bir_json):
    d = json.loads(bir_json)
    n = 0
    for f in d.get("functions", []):
        for bb in f.get("blocks", []):
            insts = bb.get("instructions", [])
            new_insts = []
            for inst in insts:
                si = inst.get("sync_info") or {}
                waits = si.get("on_wait") or []
                limit = (
                    MAX_WAITS_CTRL
                    if inst["opcode"]
                    in ("Drain", "EventSemaphore", "NoOp", "DMACopy", "DMA")
                    else MAX_WAITS_COMPUTE
                )
                if len(waits) > limit:
                    surplus = waits[:-limit]
                    for k, w in enumerate(surplus):
                        new_insts.append({
                            "name": f"{inst['name']}_wsplit{k}",
                            "engine": inst["engine"],
                            "opcode": "EventSemaphore",
                            "ins": [],
                            "outs": [],
                            "debug": inst.get("debug", 0),
                            "sync_info": {"on_update": [], "on_wait": [w]},
                        })
                        n += 1
                    si["on_wait"] = waits[-limit:]
                    inst["sync_info"] = si
                new_insts.append(inst)
            bb["instructions"] = new_insts
    return json.dumps(d).encode()


def _install_wait_split():
    global _orig_compile_bir_kernel
    if _orig_compile_bir_kernel is not None:
        return
    _orig_compile_bir_kernel = bass2jax.compile_bir_kernel

    def _patched(bir_json, tmpdir, neff_name="file.neff"):
        return _orig_compile_bir_kernel(
            _split_waits_in_bir(bir_json), tmpdir, neff_name
        )

    bass2jax.compile_bir_kernel = _patched


def build_program():
    nc = bass.Bass()
    xT = nc.declare_dram_parameter("xT", [C, TF], f16, isOutput=False)
    wqkvT = nc.declare_dram_parameter("wqkvT", [C, 3 * HD], f16, isOutput=False)
    wpT = nc.declare_dram_parameter("wpT", [HD, C], f16, isOutput=False)
    bqkv = nc.declare_dram_parameter("bqkv", [HD, 3], f32, isOutput=False)
    outT = nc.declare_dram_parameter("outT", [C, TF], f16, isOutput=True)

    with tile.TileContext(nc) as tc:
        with (
            tc.tile_pool(name="consts", bufs=1) as consts,
            tc.tile_pool(name="persist", bufs=1) as persist,
        ):
            ident = consts.tile([128, 128], f16)
            make_identity(nc, ident)
            tri = consts.tile([128, 128], f16)
            make_upper_triangular(nc, tri, val=1.0, diag=True)
            # ones row for the K=1 denominator-broadcast matmuls
            ones64 = consts.tile([1, 64], f16)
            nc.vector.memset(ones64, 1.0)
            expbias = consts.tile([128, 1], f32)
            nc.vector.memset(expbias, EXP_BIAS)

            wq_sb = consts.tile([128, NKC, 3 * HD], f16)
            nc.sync.dma_start(wq_sb, wqkvT.rearrange("(kc p) n -> p kc n", p=128))
            wp_sb = consts.tile([HD, C], f16)
            nc.sync.dma_start(wp_sb, wpT[:, :])
            b_sb = consts.tile([HD, 3], f32)
            nc.sync.dma_start(b_sb, bqkv[:, :])

            qT = persist.tile([128, TF], f16)
            kT = persist.tile([128, TF], f16)
            yT = persist.tile([128, TF], f16)
            # v in [t, hd] layout + a ones column per head for softmax sums
            v_sb = persist.tile([128, B, TKC, HL, 66], f16)
            nc.vector.memset(v_sb[:, :, :, :, 64], 1.0)

            xTr = xT.rearrange("(kc p) t -> p kc t", p=128)
            outTr = outT.rearrange("(r p) t -> p r t", p=128)

            # ---- phase 1: QKV projections (+ v transposed via XBAR DMA) ----
            with (
                tc.tile_pool(name="p1", bufs=3) as p1,
                tc.tile_pool(name="ps1", bufs=1, space="PSUM") as ps1,
            ):
                for tb in range(NTB):
                    tsl = ts(tb, 512)
                    # one 1MB DMA brings all 8 contraction chunks
                    xt = p1.tile([128, NKC, 512], f16, tag="xt", name="xt")
                    nc.sync.dma_start(xt, xTr[:, :, tsl])
                    # K-contiguous accumulation per projection; psum
                    # drains on the scalar engine (idle in phase 1)
                    for pr in range(3):
                        pst_t = ps1.tile([128, 512], f32, tag="qkvps",
                                         bufs=4, name="qkvps")
                        for kc in range(NKC):
                            nc.tensor.matmul(
                                pst_t,
                                lhsT=wq_sb[:, kc, ts(pr, HD)],
                                rhs=xt[:, kc, :],
                                start=(kc == 0),
                                stop=(kc == NKC - 1),
                            )
                        if pr == 0:
                            nc.scalar.activation(
                                qT[:, tsl], pst_t, IDENT_FN, bias=b_sb[:, 0:1]
                            )
                        elif pr == 1:
                            nc.scalar.activation(
                                kT[:, tsl], pst_t, IDENT_FN, bias=b_sb[:, 1:2]
                            )
                        else:
                            vt = p1.tile([128, 512], f16, tag="vt", name="vt")
                            nc.scalar.activation(
                                vt, pst_t, IDENT_FN, bias=b_sb[:, 2:3]
                            )
                            for i in range(4):
                                b_i, kc_i = divmod(tb * 4 + i, TKC)
                                pt = ps1.tile([128, 128], f16, tag="vtp",
                                              bufs=2, name="vtp")
                                nc.tensor.transpose(
                                    pt, vt[:, ts(i, 128)], ident
                                )
                                nc.scalar.activation(
                                    v_sb[:, b_i, kc_i, :, 0:64],
                                    pt[:, :].rearrange(
                                        "p (h d) -> p h d", h=HL
                                    ),
                                    IDENT_FN,
                                )

            # ---- phase 2: causal attention + output projection ----
            with (
                tc.tile_pool(name="p2", bufs=3) as p2,
                tc.tile_pool(name="ps2", bufs=1, space="PSUM") as ps2,
            ):
                # deferred normalize-broadcast + out-proj steps of the
                # previous block, emitted inside the next block's kc
                # loop: by then their inputs are ready, so they fill
                # exp-paced PE bubbles instead of stalling the strict
                # engine FIFOs at block boundaries
                pending = []

                def drain_pending(n):
                    for _ in range(min(n, len(pending))):
                        pending.pop(0)()

                # blocks run j-descending: the long j=3 block leads each
                # batch, giving the machine ~16 kc-steps of exp work to
                # absorb the previous batch's deferred tail
                for b_i in range(B):
                    for j in reversed(range(NQB)):
                        q_off = b_i * T + j * QB
                        qsl = slice(q_off, q_off + QB)
                        ypq = [
                            ps2.tile([65, 512], f32, tag=f"y{h}", bufs=1,
                                     name=f"ypq{h}")
                            for h in range(HL)
                        ]
                        nkc = 4 * (j + 1)

                        def emit_s(kc):
                            # both heads' S^T tiles side by side in one
                            # 2-bank psum tile -> one exp op over both;
                            # the two K=64 matmuls use disjoint PE row
                            # groups (base partitions 0/64) so they run
                            # concurrently. Columns < lo are fully
                            # masked -> never computed.
                            r = kc * 128 - j * QB
                            k_off = b_i * T + kc * 128
                            lo = max(r, 0)
                            st = ps2.tile([128, 2, 512], f32, tag="st",
                                          bufs=2, name="st")
                            for h in range(HL):
                                nc.tensor.matmul(
                                    st[:, h, lo:512],
                                    lhsT=kT[ts(h, 64), k_off:k_off + 128],
                                    rhs=qT[ts(h, 64), q_off + lo:q_off + 512],
                                    start=True,
                                    stop=True,
                                )
                            return st, lo, r

                        # software pipeline: S runs two kc-steps ahead
                        # of exp/PV so the exp stream never waits on the
                        # S->exp->PV->S round trip (PVs lag behind,
                        # buffered by the ex pool)
                        window = [emit_s(kc) for kc in range(min(2, nkc))]
                        for kc in range(nkc):
                            st, lo, r = window.pop(0)
                            ex = p2.tile([128, 2, 512], f16, tag="ex",
                                         bufs=6, name="ex")
                            nc.scalar.activation(
                                ex[:, :, lo:512], st[:, :, lo:512], EXP,
                                scale=0.125, bias=expbias,
                            )
                            if kc + 2 < nkc:
                                window.append(emit_s(kc + 2))
                            for h in range(HL):
                                if r >= 0:
                                    # causal mask on gpsimd (idle engine)
                                    nc.gpsimd.tensor_mul(
                                        ex[:, h, r:r + 128],
                                        ex[:, h, r:r + 128], tri
                                    )
                                nc.tensor.matmul(
                                    ypq[h][:, lo:512],
                                    lhsT=v_sb[:, b_i, kc, h, 0:65],
                                    rhs=ex[:, h, lo:512],
                                    start=(kc == 0),
                                    stop=(kc == nkc - 1),
                                )
                            # the norm closure (start of the recip
                            # chain) drains immediately; projection
                            # closures wait until kc>=3, by which point
                            # yT of the previous block is ready and the
                            # pp matmuls no longer stall the PE FIFO
                            drain_pending(1 if kc == 0 else
                                          (2 if kc >= 5 else 0))
                        # normalize: yT[hd, t] = yT_unnorm * (1/sumexp).
                        # ypq is drained to sbuf (f16) immediately so
                        # its psum banks free up for the next block; the
                        # reciprocal+scale runs off the psum critical
                        # path (scale on gpsimd, sbuf-only).
                        srow = p2.tile([1, 1024], f16, tag="srow", name="srow")
                        for h in range(HL):
                            nc.vector.tensor_copy(
                                srow[0:1, ts(h, 512)], ypq[h][64:65, :]
                            )
                        yraw = p2.tile([128, 512], f16, tag="yraw", name="yraw")
                        for h in range(HL):
                            nc.vector.tensor_copy(
                                yraw[ts(h, 64), :], ypq[h][0:64, :]
                            )

                        def emit_norm(srow=srow, yraw=yraw, qsl=qsl):
                            # two K=1 matmuls to disjoint PE col groups
                            # replicate each head's sums row across its
                            # 64 output partitions; one reciprocal
                            # covers both heads
                            bc = ps2.tile([128, 512], f32, tag="misc",
                                          bufs=2, name="bc")
                            for h in range(HL):
                                nc.tensor.matmul(
                                    bc[ts(h, 64), :], lhsT=ones64,
                                    rhs=srow[0:1, ts(h, 512)],
                                    start=True, stop=True,
                                )
                            rec = p2.tile([128, 512], f16, tag="rec",
                                          name="rec")
                            with nc.allow_low_precision(
                                reason="softmax 1/sum applied in f16; "
                                "2^-11 rel err is well inside the "
                                "output tolerance"
                            ):
                                nc.vector.reciprocal(rec, bc)
                                nc.gpsimd.tensor_mul(yT[:, qsl], yraw, rec)

                        def emit_proj(oc0, qsl=qsl):
                            # row-parallel output projection, two
                            # 128-row chunks per step
                            for oc in (oc0, oc0 + 1):
                                pp = ps2.tile([128, 512], f32, tag="misc",
                                              bufs=2, name="pp")
                                nc.tensor.matmul(
                                    pp,
                                    lhsT=wp_sb[:, ts(oc, 128)],
                                    rhs=yT[:, qsl],
                                    start=True,
                                    stop=True,
                                )
                                ob = p2.tile([128, 512], f16, tag="ob",
                                             bufs=6, name="ob")
                                # casts stay off the scalar engine: with
                                # the deferred tail they would insert
                                # between back-to-back exps in the
                                # scalar FIFO and pace down the exp
                                # stream
                                nc.vector.tensor_copy(ob, pp)
                                nc.sync.dma_start(outT[ts(oc, 128), qsl], ob)

                        pending.append(emit_norm)
                        for oc0 in range(0, 8, 2):
                            pending.append(
                                lambda oc0=oc0, f=emit_proj: f(oc0)
                            )
                # flush the final block's deferred tail
                drain_pending(len(pending))
    return nc


_program = None


def _get_program():
    global _program
    if _program is None:
        _install_wait_split()
        _program = build_program()
    return _program


def kernel(x, Wq, bq, Wk, bk, Wv, bv, Wp, bp):
    nc = _get_program()

    x = np.asarray(x, dtype=np.float32)
    xT = np.ascontiguousarray(x.reshape(TF, C).T.astype(NP16))
    in_maps = []
    for core in range(NCORES):
        rows = slice(core * HD, (core + 1) * HD)
        wqkvT = np.ascontiguousarray(
            np.concatenate(
                [np.asarray(W, np.float32)[rows].T for W in (Wq, Wk, Wv)], axis=1
            ).astype(NP16)
        )
        wpT = np.ascontiguousarray(np.asarray(Wp, np.float32)[:, rows].T.astype(NP16))
        bq_l = np.stack(
            [np.asarray(v, np.float32)[rows] for v in (bq, bk, bv)], axis=1
        )
        in_maps.append(
            {
                "xT": xT,
                "wqkvT": wqkvT,
                "wpT": wpT,
                "bqkv": np.ascontiguousarray(bq_l),
            }
        )

    r = bass_utils.run_bass_kernel_spmd(nc, in_maps, list(range(NCORES)))
    acc = r.results[0]["outT"].astype(np.float32)
    for core in range(1, NCORES):
        acc = acc + r.results[core]["outT"].astype(np.float32)
    out = acc.T.reshape(B, T, C) + np.asarray(bp, np.float32)[None, None, :]
    return out.astype(np.float32)
